# revision 37
# baseline (speedup 1.0000x reference)
"""Trainium2 Bass kernel for a diagonal-A linear dynamical system (LDS).

    Bu = inputs @ B            [B, T, S]
    h_t = h_{t-1} * A + Bu_t   (scan over T, diagonal A)
    y_t = h_t @ C              [B, T, O]

Shapes: inputs [16, 4096, 256], A [256], B [256, 256], C [256, 256],
h0 [256]; output float32.

Sharding: data-parallel over batch across 8 NeuronCores (2 batches per
core); A/B/C/h0 replicated.

v5 design (vs v1 which PE-transposed fp32 u):
  - u is cast to bf16 and pre-transposed to [i, t] on the host (halves
    input HBM bytes, rel err ~0.4% ≪ 2e-2 tol; layout prep like the
    B/C reshapes).  Loads are plain contiguous DMAs; no PE transposes,
    no xbar (measured only ~74 GB/s and gated the pipeline head).
  - Bu^T = B^T @ uT on PE in bf16 (1 cyc/row), accumulated over
    i-halves into PSUM.
  - DVE tensor_tensor_scan along t (fp32 internal state) chained
    across chunks: hT [s, t] in SBUF, stored bf16.
  - y[t, o] = hT_block^T @ C on PE in bf16, ACT copy PSUM->SBUF
    (fp32), DMA out per supertile.
  - ALL matmuls are bf16: mixing bf16 and fp32r matmuls in one kernel
    corrupts PE results (observed on HW: fp32_mode state interaction),
    so hT and C are bf16 rather than float32r.
"""

import ml_dtypes
import numpy as np

import concourse.bacc as bacc
import concourse.bass as bass
import concourse.mybir as mybir
import concourse.tile as tile
from concourse import bass_utils

BATCH, T, D = 16, 4096, 256
NCORES = 8
BLOC = BATCH // NCORES  # batches per core
TT = 2048               # time supertile (DMA granularity)
NSUB = TT // 128        # 128-row output subtiles per supertile
NJ = T // TT            # supertiles per sequence
SC = 1024               # scan / PSUM chunk (2 banks; scan cost is ~all
                        # fixed ~1.2us/instr, so bigger chunks = faster)
NTH = TT // SC          # chunks per supertile
MMF = 512               # matmul free size (one PSUM bank)
F32 = mybir.dt.float32
F32R = mybir.dt.float32r
BF16 = mybir.dt.bfloat16

_CACHE: dict = {}


def _build_nc():
    nc = bacc.Bacc(trn_type="TRN2", target_bir_lowering=False)

    u = nc.dram_tensor("u", [BLOC, 2, 128, T], BF16, kind="ExternalInput")  # [b, i//128, i%128, t]
    # A and h0 packed host-side: [s%128, (A cols 0:2 | h0 cols 2:4)]
    Ahd = nc.dram_tensor("Ah0", [128, 4], F32, kind="ExternalInput")
    Bd = nc.dram_tensor("B", [2, 128, D], BF16, kind="ExternalInput")  # [ihalf, i, s]
    Cd = nc.dram_tensor("C", [2, 128, D], BF16, kind="ExternalInput")  # [shalf, s, o]
    y = nc.dram_tensor("y", [BLOC, T, D], F32, kind="ExternalOutput")

    u_r = u[:].rearrange("b k p (j t) -> b j p k t", t=TT)
    # t = j*TT + c*SC + s*128 + p
    y_r = y[:].rearrange(
        "b (j c s p) o -> b j c p s o", p=128, s=SC // 128, c=NTH
    )

    mult = mybir.AluOpType.mult
    add = mybir.AluOpType.add

    with tile.TileContext(nc) as tc:
        with (
            tc.tile_pool(name="const", bufs=1) as const,
            tc.tile_pool(name="ut", bufs=BLOC * NJ) as ut_pool,
            tc.tile_pool(name="ysb", bufs=2) as ysb_pool,
            tc.tile_pool(name="hpool", bufs=1) as hpool,
            tc.tile_pool(name="ps_bu", bufs=2, space="PSUM") as ps_bu,
            tc.tile_pool(name="ps_y", bufs=4, space="PSUM") as ps_y,
        ):
            # First u supertile is the head-critical transfer: issue it
            # before everything else on the sync queue.
            uts = {}
            for b in range(BLOC):
                for j in range(NJ):
                    uts[(b, j)] = ut_pool.tile(
                        [128, 2, TT], BF16, tag="uT", name="uT"
                    )
            # Tiny consts first (they gate A_bc / scan init / LDWEIGHTS),
            # batched into single DMAs, then the first u half-supertile.
            Ah = const.tile([128, 4], F32, name="Ah")
            nc.sync.dma_start(Ah, Ahd[:])
            B_sb = const.tile([128, 2, D], BF16, name="B_sb")
            C_sb = const.tile([128, 2, D], BF16, name="C_sb")
            nc.sync.dma_start(B_sb, Bd[:].rearrange("k i s -> i k s"))
            # First chunk's first segment (t<512) lands first.
            nc.sync.dma_start(uts[(0, 0)][:, :, 0:512], u_r[0, 0][:, :, 0:512])
            nc.sync.dma_start(uts[(0, 0)][:, :, 512:SC], u_r[0, 0][:, :, 512:SC])
            nc.sync.dma_start(uts[(0, 0)][:, :, SC:TT], u_r[0, 0][:, :, SC:TT])
            nc.sync.dma_start(C_sb, Cd[:].rearrange("k i s -> i k s"))
            for b in range(BLOC):
                for j in range(NJ):
                    if (b, j) != (0, 0):
                        nc.sync.dma_start(uts[(b, j)], u_r[b, j])

            # A_bc built on DVE (idle before the scan spine); keeps ACT's
            # first-use table load off the critical path.
            ones = const.tile([128, SC], F32, name="ones")
            nc.vector.memset(ones, 1.0)
            A_bc = const.tile([128, 2, SC], F32, name="A_bc")
            for m in range(2):
                nc.vector.tensor_scalar(
                    A_bc[:, m], ones, Ah[:, m : m + 1], None, op0=mult
                )

            # hidden states, [128s, b, mhalf, t]; persistent
            hT = hpool.tile([128, BLOC, 2, T], BF16, name="hT")

            # t = s0 + s*128 + p
            y_r2 = y[:].rearrange("b (s p) o -> b p s o", p=128)

            def emit_y(b_, s0_, seg_, copy_eng):
                """y MMs + PSUM->SBUF copies + DMA out for one segment."""
                y_sb = ysb_pool.tile(
                    [128, (seg_ // 128) * D], F32, tag="y_sb", name="y_sb"
                )
                for half in range(seg_ // 256):
                    y_ps = ps_y.tile(
                        [128, 2 * D], F32, tag="y_ps", name="y_ps"
                    )
                    for i in range(2):
                        t0 = s0_ + (half * 2 + i) * 128
                        for k in range(2):
                            nc.tensor.matmul(
                                y_ps[:, i * D : (i + 1) * D],
                                hT[:, b_, k, t0 : t0 + 128],
                                C_sb[:, k],
                                start=(k == 0),
                                stop=(k == 1),
                            )
                    copy_eng(
                        y_sb[:, half * 2 * D : (half + 1) * 2 * D], y_ps
                    )
                nsub = seg_ // 128
                # y-out on the scalar HWDGE queue: doesn't contend with
                # the uT loads on sync, and drains the tail faster.
                nc.scalar.dma_start(
                    y_r2[b_][:, s0_ // 128 : s0_ // 128 + nsub],
                    y_sb.rearrange("p (s o) -> p s o", s=nsub),
                )

            # Software-pipelined: each chunk's Bu matmuls are emitted (and
            # thus prioritized on PE) BEFORE the previous segment's y-phase,
            # so the scan spine never waits on Bu behind y work.
            pending = []
            for b in range(BLOC):
                for j in range(NJ):
                    uT = uts[(b, j)]
                    for th in range(NTH):
                        c0 = j * TT + th * SC  # chunk start (abs time)
                        bu_tiles = []
                        for m in range(2):
                            bu_ps = ps_bu.tile(
                                [128, SC], F32, tag="bu_ps", name="bu_ps"
                            )
                            bu_tiles.append(bu_ps)
                            for hh in range(SC // MMF):
                                for k in range(2):
                                    nc.tensor.matmul(
                                        bu_ps[:, hh * MMF : (hh + 1) * MMF],
                                        B_sb[:, k, m * 128 : (m + 1) * 128],
                                        uT[:, k,
                                           th * SC + hh * MMF
                                           : th * SC + (hh + 1) * MMF],
                                        start=(k == 0),
                                        stop=(k == 1),
                                    )
                        for p in pending:
                            emit_y(*p, nc.scalar.copy)
                        pending = []
                        # First chunk: 2 segments so the spine starts on
                        # half the data (earlier).  Final chunk: 2 segments
                        # so the last y-phase overlaps the spine's tail.
                        last = b == BLOC - 1 and j == NJ - 1 and th == NTH - 1
                        first = b == 0 and j == 0 and th == 0
                        nseg = 2 if (last or first) else 1
                        seg = SC // nseg
                        for sg in range(nseg):
                            s0 = c0 + sg * seg
                            for m in range(2):
                                init = (
                                    Ah[:, 2 + m : 3 + m]
                                    if s0 == 0
                                    else hT[:, b, m, s0 - 1 : s0]
                                )
                                nc.vector.tensor_tensor_scan(
                                    hT[:, b, m, s0 : s0 + seg],
                                    A_bc[:, m, :seg],
                                    bu_tiles[m][:, sg * seg : (sg + 1) * seg],
                                    init,
                                    op0=mult,
                                    op1=add,
                                )
                            pending.append((b, s0, seg))
            # Tail: final segments' copies split ACT / DVE (DVE is idle).
            for idx, p in enumerate(pending):
                emit_y(*p, nc.vector.tensor_copy if idx else nc.scalar.copy)

    nc.compile()
    return nc


def _get_nc():
    if "nc" not in _CACHE:
        _CACHE["nc"] = _build_nc()
    return _CACHE["nc"]


def make_in_maps(inputs, A, B, C, h0):
    u = np.asarray(inputs, dtype=np.float32).astype(ml_dtypes.bfloat16)
    # [B, T, 256] -> [B, 2, 128, T]  (i = k*128 + p)
    u = np.ascontiguousarray(u.transpose(0, 2, 1)).reshape(BATCH, 2, 128, T)
    A2 = np.asarray(A, np.float32).reshape(2, 128).T
    h02 = np.asarray(h0, np.float32).reshape(2, 128).T
    Ah0 = np.ascontiguousarray(np.concatenate([A2, h02], axis=1))  # [128, 4]
    Br = np.ascontiguousarray(
        np.asarray(B, np.float32).reshape(2, 128, D).astype(ml_dtypes.bfloat16)
    )
    Cr = np.ascontiguousarray(
        np.asarray(C, np.float32).reshape(2, 128, D).astype(ml_dtypes.bfloat16)
    )
    return [
        {
            "u": np.ascontiguousarray(u[c * BLOC : (c + 1) * BLOC]),
            "Ah0": Ah0,
            "B": Br,
            "C": Cr,
        }
        for c in range(NCORES)
    ]


def kernel(inputs, A, B, C, h0, _trace=False):
    nc = _get_nc()
    in_maps = make_in_maps(inputs, A, B, C, h0)
    res = bass_utils.run_bass_kernel_spmd(
        nc, in_maps, core_ids=list(range(NCORES)), trace=_trace
    )
    out = np.concatenate([r["y"] for r in res.results], axis=0)
    if _trace:
        _CACHE["last_result"] = res
    return out


# revision 38
# speedup vs baseline: 1.0246x; 1.0246x over previous
"""Trainium2 Bass kernel for a diagonal-A linear dynamical system (LDS).

    Bu = inputs @ B            [B, T, S]
    h_t = h_{t-1} * A + Bu_t   (scan over T, diagonal A)
    y_t = h_t @ C              [B, T, O]

Shapes: inputs [16, 4096, 256], A [256], B [256, 256], C [256, 256],
h0 [256]; output float32.

Sharding: data-parallel over batch across 8 NeuronCores (2 batches per
core); A/B/C/h0 replicated.

v5 design (vs v1 which PE-transposed fp32 u):
  - u is cast to bf16 and pre-transposed to [i, t] on the host (halves
    input HBM bytes, rel err ~0.4% ≪ 2e-2 tol; layout prep like the
    B/C reshapes).  Loads are plain contiguous DMAs; no PE transposes,
    no xbar (measured only ~74 GB/s and gated the pipeline head).
  - Bu^T = B^T @ uT on PE in bf16 (1 cyc/row), accumulated over
    i-halves into PSUM.
  - DVE tensor_tensor_scan along t (fp32 internal state) chained
    across chunks: hT [s, t] in SBUF, stored bf16.
  - y[t, o] = hT_block^T @ C on PE in bf16, ACT copy PSUM->SBUF
    (fp32), DMA out per supertile.
  - ALL matmuls are bf16: mixing bf16 and fp32r matmuls in one kernel
    corrupts PE results (observed on HW: fp32_mode state interaction),
    so hT and C are bf16 rather than float32r.
"""

import ml_dtypes
import numpy as np

import concourse.bacc as bacc
import concourse.bass as bass
import concourse.mybir as mybir
import concourse.tile as tile
from concourse import bass_utils

BATCH, T, D = 16, 4096, 256
NCORES = 8
BLOC = BATCH // NCORES  # batches per core
TT = 2048               # time supertile (DMA granularity)
NSUB = TT // 128        # 128-row output subtiles per supertile
NJ = T // TT            # supertiles per sequence
SC = 1024               # scan / PSUM chunk (2 banks; scan cost is ~all
                        # fixed ~1.2us/instr, so bigger chunks = faster)
NTH = TT // SC          # chunks per supertile
MMF = 512               # matmul free size (one PSUM bank)
F32 = mybir.dt.float32
F32R = mybir.dt.float32r
BF16 = mybir.dt.bfloat16

_CACHE: dict = {}


def _build_nc():
    nc = bacc.Bacc(trn_type="TRN2", target_bir_lowering=False)

    u = nc.dram_tensor("u", [BLOC, 2, 128, T], BF16, kind="ExternalInput")  # [b, i//128, i%128, t]
    # A and h0 packed host-side: [s%128, (A cols 0:2 | h0 cols 2:4)]
    Ahd = nc.dram_tensor("Ah0", [128, 4], F32, kind="ExternalInput")
    Bd = nc.dram_tensor("B", [2, 128, D], BF16, kind="ExternalInput")  # [ihalf, i, s]
    Cd = nc.dram_tensor("C", [2, 128, D], BF16, kind="ExternalInput")  # [shalf, s, o]
    y = nc.dram_tensor("y", [BLOC, T, D], F32, kind="ExternalOutput")

    u_r = u[:].rearrange("b k p (j t) -> b j p k t", t=TT)
    # t = j*TT + c*SC + s*128 + p
    y_r = y[:].rearrange(
        "b (j c s p) o -> b j c p s o", p=128, s=SC // 128, c=NTH
    )

    mult = mybir.AluOpType.mult
    add = mybir.AluOpType.add

    with tile.TileContext(nc) as tc:
        with (
            tc.tile_pool(name="const", bufs=1) as const,
            tc.tile_pool(name="ut", bufs=BLOC * NJ) as ut_pool,
            tc.tile_pool(name="ysb", bufs=2) as ysb_pool,
            tc.tile_pool(name="hpool", bufs=1) as hpool,
            tc.tile_pool(name="ps_bu", bufs=2, space="PSUM") as ps_bu,
            tc.tile_pool(name="ps_y", bufs=4, space="PSUM") as ps_y,
        ):
            # First u supertile is the head-critical transfer: issue it
            # before everything else on the sync queue.
            uts = {}
            for b in range(BLOC):
                for j in range(NJ):
                    uts[(b, j)] = ut_pool.tile(
                        [128, 2, TT], BF16, tag="uT", name="uT"
                    )
            # Tiny consts first (they gate A_bc / scan init / LDWEIGHTS),
            # batched into single DMAs, then the first u half-supertile.
            Ah = const.tile([128, 4], F32, name="Ah")
            nc.sync.dma_start(Ah, Ahd[:])
            B_sb = const.tile([128, 2, D], BF16, name="B_sb")
            C_sb = const.tile([128, 2, D], BF16, name="C_sb")
            nc.sync.dma_start(B_sb, Bd[:].rearrange("k i s -> i k s"))
            # First chunk's first segment (t<512) lands first.
            nc.sync.dma_start(uts[(0, 0)][:, :, 0:512], u_r[0, 0][:, :, 0:512])
            nc.sync.dma_start(uts[(0, 0)][:, :, 512:SC], u_r[0, 0][:, :, 512:SC])
            nc.sync.dma_start(uts[(0, 0)][:, :, SC:TT], u_r[0, 0][:, :, SC:TT])
            nc.sync.dma_start(C_sb, Cd[:].rearrange("k i s -> i k s"))
            for b in range(BLOC):
                for j in range(NJ):
                    if (b, j) != (0, 0):
                        nc.sync.dma_start(uts[(b, j)], u_r[b, j])

            # A_bc built on DVE (idle before the scan spine); keeps ACT's
            # first-use table load off the critical path.
            ones = const.tile([128, SC], F32, name="ones")
            nc.vector.memset(ones, 1.0)
            A_bc = const.tile([128, 2, SC], F32, name="A_bc")
            for m in range(2):
                nc.vector.tensor_scalar(
                    A_bc[:, m], ones, Ah[:, m : m + 1], None, op0=mult
                )

            # hidden states, [128s, b, mhalf, t]; persistent
            hT = hpool.tile([128, BLOC, 2, T], BF16, name="hT")

            # t = s0 + s*128 + p
            y_r2 = y[:].rearrange("b (s p) o -> b p s o", p=128)

            def emit_y(b_, s0_, seg_, copy_eng):
                """y MMs + PSUM->SBUF copies + DMA out for one segment."""
                y_sb = ysb_pool.tile(
                    [128, (seg_ // 128) * D], F32, tag="y_sb", name="y_sb"
                )
                for half in range(seg_ // 256):
                    y_ps = ps_y.tile(
                        [128, 2 * D], F32, tag="y_ps", name="y_ps"
                    )
                    for i in range(2):
                        t0 = s0_ + (half * 2 + i) * 128
                        for k in range(2):
                            nc.tensor.matmul(
                                y_ps[:, i * D : (i + 1) * D],
                                hT[:, b_, k, t0 : t0 + 128],
                                C_sb[:, k],
                                start=(k == 0),
                                stop=(k == 1),
                            )
                    copy_eng(
                        y_sb[:, half * 2 * D : (half + 1) * 2 * D], y_ps
                    )
                nsub = seg_ // 128
                nc.sync.dma_start(
                    y_r2[b_][:, s0_ // 128 : s0_ // 128 + nsub],
                    y_sb.rearrange("p (s o) -> p s o", s=nsub),
                )

            # Software-pipelined: each chunk's Bu matmuls are emitted (and
            # thus prioritized on PE) BEFORE the previous segment's y-phase,
            # so the scan spine never waits on Bu behind y work.
            pending = []
            for b in range(BLOC):
                for j in range(NJ):
                    uT = uts[(b, j)]
                    for th in range(NTH):
                        c0 = j * TT + th * SC  # chunk start (abs time)
                        bu_tiles = []
                        for m in range(2):
                            bu_ps = ps_bu.tile(
                                [128, SC], F32, tag="bu_ps", name="bu_ps"
                            )
                            bu_tiles.append(bu_ps)
                            for hh in range(SC // MMF):
                                for k in range(2):
                                    nc.tensor.matmul(
                                        bu_ps[:, hh * MMF : (hh + 1) * MMF],
                                        B_sb[:, k, m * 128 : (m + 1) * 128],
                                        uT[:, k,
                                           th * SC + hh * MMF
                                           : th * SC + (hh + 1) * MMF],
                                        start=(k == 0),
                                        stop=(k == 1),
                                    )
                        for p in pending:
                            emit_y(*p, nc.scalar.copy)
                        pending = []
                        # First chunk: 2 segments so the spine starts on
                        # half the data (earlier).  Final chunk: 2 segments
                        # so the last y-phase overlaps the spine's tail.
                        last = b == BLOC - 1 and j == NJ - 1 and th == NTH - 1
                        first = b == 0 and j == 0 and th == 0
                        nseg = 2 if (last or first) else 1
                        seg = SC // nseg
                        for sg in range(nseg):
                            s0 = c0 + sg * seg
                            for m in range(2):
                                init = (
                                    Ah[:, 2 + m : 3 + m]
                                    if s0 == 0
                                    else hT[:, b, m, s0 - 1 : s0]
                                )
                                nc.vector.tensor_tensor_scan(
                                    hT[:, b, m, s0 : s0 + seg],
                                    A_bc[:, m, :seg],
                                    bu_tiles[m][:, sg * seg : (sg + 1) * seg],
                                    init,
                                    op0=mult,
                                    op1=add,
                                )
                            pending.append((b, s0, seg))
            # Tail: final segments' copies split ACT / DVE (DVE is idle).
            for idx, p in enumerate(pending):
                emit_y(*p, nc.vector.tensor_copy if idx else nc.scalar.copy)

    nc.compile()
    return nc


def _get_nc():
    if "nc" not in _CACHE:
        _CACHE["nc"] = _build_nc()
    return _CACHE["nc"]


def make_in_maps(inputs, A, B, C, h0):
    u = np.asarray(inputs, dtype=np.float32).astype(ml_dtypes.bfloat16)
    # [B, T, 256] -> [B, 2, 128, T]  (i = k*128 + p)
    u = np.ascontiguousarray(u.transpose(0, 2, 1)).reshape(BATCH, 2, 128, T)
    A2 = np.asarray(A, np.float32).reshape(2, 128).T
    h02 = np.asarray(h0, np.float32).reshape(2, 128).T
    Ah0 = np.ascontiguousarray(np.concatenate([A2, h02], axis=1))  # [128, 4]
    Br = np.ascontiguousarray(
        np.asarray(B, np.float32).reshape(2, 128, D).astype(ml_dtypes.bfloat16)
    )
    Cr = np.ascontiguousarray(
        np.asarray(C, np.float32).reshape(2, 128, D).astype(ml_dtypes.bfloat16)
    )
    return [
        {
            "u": np.ascontiguousarray(u[c * BLOC : (c + 1) * BLOC]),
            "Ah0": Ah0,
            "B": Br,
            "C": Cr,
        }
        for c in range(NCORES)
    ]


def kernel(inputs, A, B, C, h0, _trace=False):
    nc = _get_nc()
    in_maps = make_in_maps(inputs, A, B, C, h0)
    res = bass_utils.run_bass_kernel_spmd(
        nc, in_maps, core_ids=list(range(NCORES)), trace=_trace
    )
    out = np.concatenate([r["y"] for r in res.results], axis=0)
    if _trace:
        _CACHE["last_result"] = res
    return out


# revision 40
# speedup vs baseline: 1.1077x; 1.0811x over previous
"""Trainium2 Bass kernel for a diagonal-A linear dynamical system (LDS).

    Bu = inputs @ B            [B, T, S]
    h_t = h_{t-1} * A + Bu_t   (scan over T, diagonal A)
    y_t = h_t @ C              [B, T, O]

Shapes: inputs [16, 4096, 256], A [256], B [256, 256], C [256, 256],
h0 [256]; output float32.

Sharding: data-parallel over batch across 8 NeuronCores (2 batches per
core); A/B/C/h0 replicated.

v5 design (vs v1 which PE-transposed fp32 u):
  - u is cast to bf16 and pre-transposed to [i, t] on the host (halves
    input HBM bytes, rel err ~0.4% ≪ 2e-2 tol; layout prep like the
    B/C reshapes).  Loads are plain contiguous DMAs; no PE transposes,
    no xbar (measured only ~74 GB/s and gated the pipeline head).
  - Bu^T = B^T @ uT on PE in bf16 (1 cyc/row), accumulated over
    i-halves into PSUM.
  - DVE tensor_tensor_scan along t (fp32 internal state) chained
    across chunks: hT [s, t] in SBUF, stored bf16.
  - y[t, o] = hT_block^T @ C on PE in bf16, ACT copy PSUM->SBUF
    (fp32), DMA out per supertile.
  - ALL matmuls are bf16: mixing bf16 and fp32r matmuls in one kernel
    corrupts PE results (observed on HW: fp32_mode state interaction),
    so hT and C are bf16 rather than float32r.
"""

import ml_dtypes
import numpy as np

import concourse.bacc as bacc
import concourse.bass as bass
import concourse.mybir as mybir
import concourse.tile as tile
from concourse import bass_utils

BATCH, T, D = 16, 4096, 256
NCORES = 8
BLOC = BATCH // NCORES  # batches per core
TT = 2048               # time supertile (DMA granularity)
NSUB = TT // 128        # 128-row output subtiles per supertile
NJ = T // TT            # supertiles per sequence
SC = 1024               # scan / PSUM chunk (2 banks; scan cost is ~all
                        # fixed ~1.2us/instr, so bigger chunks = faster)
NTH = TT // SC          # chunks per supertile
MMF = 512               # matmul free size (one PSUM bank)
F32 = mybir.dt.float32
F32R = mybir.dt.float32r
BF16 = mybir.dt.bfloat16

_CACHE: dict = {}


def _build_nc():
    nc = bacc.Bacc(trn_type="TRN2", target_bir_lowering=False)

    u = nc.dram_tensor("u", [BLOC, 2, 128, T], BF16, kind="ExternalInput")  # [b, i//128, i%128, t]
    # A and h0 packed host-side: [s%128, (A cols 0:2 | h0 cols 2:4)]
    Ahd = nc.dram_tensor("Ah0", [128, 4], F32, kind="ExternalInput")
    Bd = nc.dram_tensor("B", [2, 128, D], BF16, kind="ExternalInput")  # [ihalf, i, s]
    Cd = nc.dram_tensor("C", [2, 128, D], BF16, kind="ExternalInput")  # [shalf, s, o]
    y = nc.dram_tensor("y", [BLOC, T, D], F32, kind="ExternalOutput")

    u_r = u[:].rearrange("b k p (j t) -> b j p k t", t=TT)
    # t = j*TT + c*SC + s*128 + p
    y_r = y[:].rearrange(
        "b (j c s p) o -> b j c p s o", p=128, s=SC // 128, c=NTH
    )

    mult = mybir.AluOpType.mult
    add = mybir.AluOpType.add

    with tile.TileContext(nc) as tc:
        with (
            tc.tile_pool(name="const", bufs=1) as const,
            tc.tile_pool(name="ut", bufs=BLOC * NJ) as ut_pool,
            tc.tile_pool(name="ysb", bufs=2) as ysb_pool,
            tc.tile_pool(name="hpool", bufs=1) as hpool,
            tc.tile_pool(name="ps_bu", bufs=2, space="PSUM") as ps_bu,
            tc.tile_pool(name="ps_y", bufs=4, space="PSUM") as ps_y,
        ):
            # First u supertile is the head-critical transfer: issue it
            # before everything else on the sync queue.
            uts = {}
            for b in range(BLOC):
                for j in range(NJ):
                    uts[(b, j)] = ut_pool.tile(
                        [128, 2, TT], BF16, tag="uT", name="uT"
                    )
            # Tiny consts first (they gate A_bc / scan init / LDWEIGHTS),
            # batched into single DMAs, then the first u half-supertile.
            Ah = const.tile([128, 4], F32, name="Ah")
            nc.sync.dma_start(Ah, Ahd[:])
            B_sb = const.tile([128, 2, D], BF16, name="B_sb")
            C_sb = const.tile([128, 2, D], BF16, name="C_sb")
            nc.sync.dma_start(B_sb, Bd[:].rearrange("k i s -> i k s"))
            nc.sync.dma_start(uts[(0, 0)][:, :, 0:SC], u_r[0, 0][:, :, 0:SC])
            nc.sync.dma_start(uts[(0, 0)][:, :, SC:TT], u_r[0, 0][:, :, SC:TT])
            nc.sync.dma_start(C_sb, Cd[:].rearrange("k i s -> i k s"))
            for b in range(BLOC):
                for j in range(NJ):
                    if (b, j) != (0, 0):
                        nc.sync.dma_start(uts[(b, j)], u_r[b, j])

            # A_bc built on DVE (idle before the scan spine); keeps ACT's
            # first-use table load off the critical path.
            ones = const.tile([128, SC], F32, name="ones")
            nc.vector.memset(ones, 1.0)
            A_bc = const.tile([128, 2, SC], F32, name="A_bc")
            for m in range(2):
                nc.vector.tensor_scalar(
                    A_bc[:, m], ones, Ah[:, m : m + 1], None, op0=mult
                )

            # hidden states, [128s, b, mhalf, t]; persistent
            hT = hpool.tile([128, BLOC, 2, T], BF16, name="hT")

            # t = s0 + s*128 + p
            y_r2 = y[:].rearrange("b (s p) o -> b p s o", p=128)

            def emit_y(b_, s0_, seg_, copy_eng):
                """y MMs + PSUM->SBUF copies + DMA out for one segment."""
                y_sb = ysb_pool.tile(
                    [128, (seg_ // 128) * D], F32, tag="y_sb", name="y_sb"
                )
                for half in range(seg_ // 256):
                    y_ps = ps_y.tile(
                        [128, 2 * D], F32, tag="y_ps", name="y_ps"
                    )
                    for i in range(2):
                        t0 = s0_ + (half * 2 + i) * 128
                        for k in range(2):
                            nc.tensor.matmul(
                                y_ps[:, i * D : (i + 1) * D],
                                hT[:, b_, k, t0 : t0 + 128],
                                C_sb[:, k],
                                start=(k == 0),
                                stop=(k == 1),
                            )
                    copy_eng(
                        y_sb[:, half * 2 * D : (half + 1) * 2 * D], y_ps
                    )
                nsub = seg_ // 128
                nc.sync.dma_start(
                    y_r2[b_][:, s0_ // 128 : s0_ // 128 + nsub],
                    y_sb.rearrange("p (s o) -> p s o", s=nsub),
                )

            # Software-pipelined: each chunk's Bu matmuls are emitted (and
            # thus prioritized on PE) BEFORE the previous segment's y-phase,
            # so the scan spine never waits on Bu behind y work.
            pending = []
            for b in range(BLOC):
                for j in range(NJ):
                    uT = uts[(b, j)]
                    for th in range(NTH):
                        c0 = j * TT + th * SC  # chunk start (abs time)
                        bu_tiles = []
                        for m in range(2):
                            bu_ps = ps_bu.tile(
                                [128, SC], F32, tag="bu_ps", name="bu_ps"
                            )
                            bu_tiles.append(bu_ps)
                            for hh in range(SC // MMF):
                                for k in range(2):
                                    nc.tensor.matmul(
                                        bu_ps[:, hh * MMF : (hh + 1) * MMF],
                                        B_sb[:, k, m * 128 : (m + 1) * 128],
                                        uT[:, k,
                                           th * SC + hh * MMF
                                           : th * SC + (hh + 1) * MMF],
                                        start=(k == 0),
                                        stop=(k == 1),
                                    )
                        for p in pending:
                            emit_y(*p, nc.scalar.copy)
                        pending = []
                        # Final chunk: 2 segments of SC//2 so the last
                        # y-phase overlaps the tail of the scan spine.
                        last = b == BLOC - 1 and j == NJ - 1 and th == NTH - 1
                        nseg = 2 if last else 1
                        seg = SC // nseg
                        for sg in range(nseg):
                            s0 = c0 + sg * seg
                            for m in range(2):
                                init = (
                                    Ah[:, 2 + m : 3 + m]
                                    if s0 == 0
                                    else hT[:, b, m, s0 - 1 : s0]
                                )
                                nc.vector.tensor_tensor_scan(
                                    hT[:, b, m, s0 : s0 + seg],
                                    A_bc[:, m, :seg],
                                    bu_tiles[m][:, sg * seg : (sg + 1) * seg],
                                    init,
                                    op0=mult,
                                    op1=add,
                                )
                            pending.append((b, s0, seg))
            # Tail: final segments' copies split ACT / DVE (DVE is idle).
            for idx, p in enumerate(pending):
                emit_y(*p, nc.vector.tensor_copy if idx else nc.scalar.copy)

    nc.compile()
    return nc


def _get_nc():
    if "nc" not in _CACHE:
        _CACHE["nc"] = _build_nc()
    return _CACHE["nc"]


def make_in_maps(inputs, A, B, C, h0):
    u = np.asarray(inputs, dtype=np.float32).astype(ml_dtypes.bfloat16)
    # [B, T, 256] -> [B, 2, 128, T]  (i = k*128 + p)
    u = np.ascontiguousarray(u.transpose(0, 2, 1)).reshape(BATCH, 2, 128, T)
    A2 = np.asarray(A, np.float32).reshape(2, 128).T
    h02 = np.asarray(h0, np.float32).reshape(2, 128).T
    Ah0 = np.ascontiguousarray(np.concatenate([A2, h02], axis=1))  # [128, 4]
    Br = np.ascontiguousarray(
        np.asarray(B, np.float32).reshape(2, 128, D).astype(ml_dtypes.bfloat16)
    )
    Cr = np.ascontiguousarray(
        np.asarray(C, np.float32).reshape(2, 128, D).astype(ml_dtypes.bfloat16)
    )
    return [
        {
            "u": np.ascontiguousarray(u[c * BLOC : (c + 1) * BLOC]),
            "Ah0": Ah0,
            "B": Br,
            "C": Cr,
        }
        for c in range(NCORES)
    ]


def kernel(inputs, A, B, C, h0, _trace=False):
    nc = _get_nc()
    in_maps = make_in_maps(inputs, A, B, C, h0)
    res = bass_utils.run_bass_kernel_spmd(
        nc, in_maps, core_ids=list(range(NCORES)), trace=_trace
    )
    out = np.concatenate([r["y"] for r in res.results], axis=0)
    if _trace:
        _CACHE["last_result"] = res
    return out


# revision 43
# speedup vs baseline: 1.1593x; 1.0466x over previous
"""Trainium2 Bass kernel for a diagonal-A linear dynamical system (LDS).

    Bu = inputs @ B            [B, T, S]
    h_t = h_{t-1} * A + Bu_t   (scan over T, diagonal A)
    y_t = h_t @ C              [B, T, O]

Shapes: inputs [16, 4096, 256], A [256], B [256, 256], C [256, 256],
h0 [256]; output float32.

Sharding: data-parallel over batch across 8 NeuronCores (2 batches per
core); A/B/C/h0 replicated.

v5 design (vs v1 which PE-transposed fp32 u):
  - u is cast to bf16 and pre-transposed to [i, t] on the host (halves
    input HBM bytes, rel err ~0.4% ≪ 2e-2 tol; layout prep like the
    B/C reshapes).  Loads are plain contiguous DMAs; no PE transposes,
    no xbar (measured only ~74 GB/s and gated the pipeline head).
  - Bu^T = B^T @ uT on PE in bf16 (1 cyc/row), accumulated over
    i-halves into PSUM.
  - DVE tensor_tensor_scan along t (fp32 internal state) chained
    across chunks: hT [s, t] in SBUF, stored bf16.
  - y[t, o] = hT_block^T @ C on PE in bf16, ACT copy PSUM->SBUF
    (fp32), DMA out per supertile.
  - ALL matmuls are bf16: mixing bf16 and fp32r matmuls in one kernel
    corrupts PE results (observed on HW: fp32_mode state interaction),
    so hT and C are bf16 rather than float32r.
"""

import ml_dtypes
import numpy as np

import concourse.bacc as bacc
import concourse.bass as bass
import concourse.mybir as mybir
import concourse.tile as tile
from concourse import bass_utils

BATCH, T, D = 16, 4096, 256
NCORES = 8
BLOC = BATCH // NCORES  # batches per core
TT = 2048               # time supertile (DMA granularity)
NSUB = TT // 128        # 128-row output subtiles per supertile
NJ = T // TT            # supertiles per sequence
SC = 1024               # scan / PSUM chunk (2 banks; scan cost is ~all
                        # fixed ~1.2us/instr, so bigger chunks = faster)
NTH = TT // SC          # chunks per supertile
MMF = 512               # matmul free size (one PSUM bank)
F32 = mybir.dt.float32
F32R = mybir.dt.float32r
BF16 = mybir.dt.bfloat16

_CACHE: dict = {}


def _build_nc():
    nc = bacc.Bacc(trn_type="TRN2", target_bir_lowering=False)

    u = nc.dram_tensor("u", [BLOC, 2, 128, T], BF16, kind="ExternalInput")  # [b, i//128, i%128, t]
    # A and h0 packed host-side: [s%128, (A cols 0:2 | h0 cols 2:4)]
    Ahd = nc.dram_tensor("Ah0", [128, 4], F32, kind="ExternalInput")
    Bd = nc.dram_tensor("B", [2, 128, D], BF16, kind="ExternalInput")  # [ihalf, i, s]
    Cd = nc.dram_tensor("C", [2, 128, D], BF16, kind="ExternalInput")  # [shalf, s, o]
    y = nc.dram_tensor("y", [BLOC, T, D], F32, kind="ExternalOutput")

    u_r = u[:].rearrange("b k p (j t) -> b j p k t", t=TT)
    # t = j*TT + c*SC + s*128 + p
    y_r = y[:].rearrange(
        "b (j c s p) o -> b j c p s o", p=128, s=SC // 128, c=NTH
    )

    mult = mybir.AluOpType.mult
    add = mybir.AluOpType.add

    with tile.TileContext(nc) as tc:
        with (
            tc.tile_pool(name="const", bufs=1) as const,
            tc.tile_pool(name="ut", bufs=BLOC * NJ) as ut_pool,
            tc.tile_pool(name="ysb", bufs=4) as ysb_pool,
            tc.tile_pool(name="hpool", bufs=1) as hpool,
            tc.tile_pool(name="ps_bu", bufs=2, space="PSUM") as ps_bu,
            tc.tile_pool(name="ps_y", bufs=4, space="PSUM") as ps_y,
        ):
            # First u supertile is the head-critical transfer: issue it
            # before everything else on the sync queue.
            uts = {}
            for b in range(BLOC):
                for j in range(NJ):
                    uts[(b, j)] = ut_pool.tile(
                        [128, 2, TT], BF16, tag="uT", name="uT"
                    )
            # Tiny consts first (they gate A_bc / scan init / LDWEIGHTS),
            # batched into single DMAs, then the first u half-supertile.
            Ah = const.tile([128, 4], F32, name="Ah")
            nc.sync.dma_start(Ah, Ahd[:])
            B_sb = const.tile([128, 2, D], BF16, name="B_sb")
            C_sb = const.tile([128, 2, D], BF16, name="C_sb")
            nc.sync.dma_start(B_sb, Bd[:].rearrange("k i s -> i k s"))
            # First chunk's first segment (t<512) lands first.
            nc.sync.dma_start(uts[(0, 0)][:, :, 0:512], u_r[0, 0][:, :, 0:512])
            nc.sync.dma_start(uts[(0, 0)][:, :, 512:SC], u_r[0, 0][:, :, 512:SC])
            nc.sync.dma_start(uts[(0, 0)][:, :, SC:TT], u_r[0, 0][:, :, SC:TT])
            nc.sync.dma_start(C_sb, Cd[:].rearrange("k i s -> i k s"))
            for b in range(BLOC):
                for j in range(NJ):
                    if (b, j) != (0, 0):
                        nc.sync.dma_start(uts[(b, j)], u_r[b, j])

            # A_bc built on DVE (idle before the scan spine); keeps ACT's
            # first-use table load off the critical path.
            ones = const.tile([128, SC], F32, name="ones")
            nc.vector.memset(ones, 1.0)
            A_bc = const.tile([128, 2, SC], F32, name="A_bc")
            for m in range(2):
                nc.vector.tensor_scalar(
                    A_bc[:, m], ones, Ah[:, m : m + 1], None, op0=mult
                )

            # hidden states, [128s, b, mhalf, t]; persistent
            hT = hpool.tile([128, BLOC, 2, T], BF16, name="hT")

            # t = s0 + s*128 + p
            y_r2 = y[:].rearrange("b (s p) o -> b p s o", p=128)

            def emit_y(b_, s0_, seg_, copy_eng):
                """y MMs + PSUM->SBUF copies + DMA out for one segment."""
                y_sb = ysb_pool.tile(
                    [128, (seg_ // 128) * D], F32, tag="y_sb", name="y_sb"
                )
                for half in range(seg_ // 256):
                    y_ps = ps_y.tile(
                        [128, 2 * D], F32, tag="y_ps", name="y_ps"
                    )
                    for i in range(2):
                        t0 = s0_ + (half * 2 + i) * 128
                        for k in range(2):
                            nc.tensor.matmul(
                                y_ps[:, i * D : (i + 1) * D],
                                hT[:, b_, k, t0 : t0 + 128],
                                C_sb[:, k],
                                start=(k == 0),
                                stop=(k == 1),
                            )
                    copy_eng(
                        y_sb[:, half * 2 * D : (half + 1) * 2 * D], y_ps
                    )
                nsub = seg_ // 128
                nc.sync.dma_start(
                    y_r2[b_][:, s0_ // 128 : s0_ // 128 + nsub],
                    y_sb.rearrange("p (s o) -> p s o", s=nsub),
                )

            # Software-pipelined: each chunk's Bu matmuls are emitted (and
            # thus prioritized on PE) BEFORE the previous segment's y-phase,
            # so the scan spine never waits on Bu behind y work.
            pending = []
            for b in range(BLOC):
                for j in range(NJ):
                    uT = uts[(b, j)]
                    for th in range(NTH):
                        c0 = j * TT + th * SC  # chunk start (abs time)
                        bu_tiles = []
                        for m in range(2):
                            bu_ps = ps_bu.tile(
                                [128, SC], F32, tag="bu_ps", name="bu_ps"
                            )
                            bu_tiles.append(bu_ps)
                            for hh in range(SC // MMF):
                                for k in range(2):
                                    nc.tensor.matmul(
                                        bu_ps[:, hh * MMF : (hh + 1) * MMF],
                                        B_sb[:, k, m * 128 : (m + 1) * 128],
                                        uT[:, k,
                                           th * SC + hh * MMF
                                           : th * SC + (hh + 1) * MMF],
                                        start=(k == 0),
                                        stop=(k == 1),
                                    )
                        for p in pending:
                            emit_y(*p, nc.scalar.copy)
                        pending = []
                        # First chunk: 2 segments so the spine starts on
                        # half the data.  Final chunk: 2 segments so the
                        # last y-phase overlaps the spine's tail.
                        last = b == BLOC - 1 and j == NJ - 1 and th == NTH - 1
                        first = b == 0 and j == 0 and th == 0
                        nseg = 2 if (last or first) else 1
                        seg = SC // nseg
                        for sg in range(nseg):
                            s0 = c0 + sg * seg
                            for m in range(2):
                                init = (
                                    Ah[:, 2 + m : 3 + m]
                                    if s0 == 0
                                    else hT[:, b, m, s0 - 1 : s0]
                                )
                                nc.vector.tensor_tensor_scan(
                                    hT[:, b, m, s0 : s0 + seg],
                                    A_bc[:, m, :seg],
                                    bu_tiles[m][:, sg * seg : (sg + 1) * seg],
                                    init,
                                    op0=mult,
                                    op1=add,
                                )
                            pending.append((b, s0, seg))
            # Tail: final segments' copies split ACT / DVE (DVE is idle).
            for idx, p in enumerate(pending):
                emit_y(*p, nc.vector.tensor_copy if idx else nc.scalar.copy)

    nc.compile()
    return nc


def _get_nc():
    if "nc" not in _CACHE:
        _CACHE["nc"] = _build_nc()
    return _CACHE["nc"]


def make_in_maps(inputs, A, B, C, h0):
    u = np.asarray(inputs, dtype=np.float32).astype(ml_dtypes.bfloat16)
    # [B, T, 256] -> [B, 2, 128, T]  (i = k*128 + p)
    u = np.ascontiguousarray(u.transpose(0, 2, 1)).reshape(BATCH, 2, 128, T)
    A2 = np.asarray(A, np.float32).reshape(2, 128).T
    h02 = np.asarray(h0, np.float32).reshape(2, 128).T
    Ah0 = np.ascontiguousarray(np.concatenate([A2, h02], axis=1))  # [128, 4]
    Br = np.ascontiguousarray(
        np.asarray(B, np.float32).reshape(2, 128, D).astype(ml_dtypes.bfloat16)
    )
    Cr = np.ascontiguousarray(
        np.asarray(C, np.float32).reshape(2, 128, D).astype(ml_dtypes.bfloat16)
    )
    return [
        {
            "u": np.ascontiguousarray(u[c * BLOC : (c + 1) * BLOC]),
            "Ah0": Ah0,
            "B": Br,
            "C": Cr,
        }
        for c in range(NCORES)
    ]


def kernel(inputs, A, B, C, h0, _trace=False):
    nc = _get_nc()
    in_maps = make_in_maps(inputs, A, B, C, h0)
    res = bass_utils.run_bass_kernel_spmd(
        nc, in_maps, core_ids=list(range(NCORES)), trace=_trace
    )
    out = np.concatenate([r["y"] for r in res.results], axis=0)
    if _trace:
        _CACHE["last_result"] = res
    return out


# revision 51
# speedup vs baseline: 1.1975x; 1.0329x over previous
"""Trainium2 Bass kernel for a diagonal-A linear dynamical system (LDS).

    Bu = inputs @ B            [B, T, S]
    h_t = h_{t-1} * A + Bu_t   (scan over T, diagonal A)
    y_t = h_t @ C              [B, T, O]

Shapes: inputs [16, 4096, 256], A [256], B [256, 256], C [256, 256],
h0 [256]; output float32.

Sharding: data-parallel over batch across 8 NeuronCores (2 batches per
core); A/B/C/h0 replicated.

v5 design (vs v1 which PE-transposed fp32 u):
  - u is cast to bf16 and pre-transposed to [i, t] on the host (halves
    input HBM bytes, rel err ~0.4% ≪ 2e-2 tol; layout prep like the
    B/C reshapes).  Loads are plain contiguous DMAs; no PE transposes,
    no xbar (measured only ~74 GB/s and gated the pipeline head).
  - Bu^T = B^T @ uT on PE in bf16 (1 cyc/row), accumulated over
    i-halves into PSUM.
  - DVE tensor_tensor_scan along t (fp32 internal state) chained
    across chunks: hT [s, t] in SBUF, stored bf16.
  - y[t, o] = hT_block^T @ C on PE in bf16, ACT copy PSUM->SBUF
    (fp32), DMA out per supertile.
  - ALL matmuls are bf16: mixing bf16 and fp32r matmuls in one kernel
    corrupts PE results (observed on HW: fp32_mode state interaction),
    so hT and C are bf16 rather than float32r.
"""

import ml_dtypes
import numpy as np

import concourse.bacc as bacc
import concourse.bass as bass
import concourse.mybir as mybir
import concourse.tile as tile
from concourse import bass_utils

BATCH, T, D = 16, 4096, 256
NCORES = 8
BLOC = BATCH // NCORES  # batches per core
TT = 2048               # time supertile (DMA granularity)
NSUB = TT // 128        # 128-row output subtiles per supertile
NJ = T // TT            # supertiles per sequence
SC = 1024               # scan / PSUM chunk (2 banks; scan cost is ~all
                        # fixed ~1.2us/instr, so bigger chunks = faster)
NTH = TT // SC          # chunks per supertile
MMF = 512               # matmul free size (one PSUM bank)
F32 = mybir.dt.float32
F32R = mybir.dt.float32r
BF16 = mybir.dt.bfloat16

_CACHE: dict = {}


def _build_nc():
    nc = bacc.Bacc(trn_type="TRN2", target_bir_lowering=False)

    u = nc.dram_tensor("u", [BLOC, 2, 128, T], BF16, kind="ExternalInput")  # [b, i//128, i%128, t]
    # A and h0 packed host-side: [s%128, (A cols 0:2 | h0 cols 2:4)]
    Ahd = nc.dram_tensor("Ah0", [128, 4], F32, kind="ExternalInput")
    Bd = nc.dram_tensor("B", [2, 128, D], BF16, kind="ExternalInput")  # [ihalf, i, s]
    Cd = nc.dram_tensor("C", [2, 128, D], BF16, kind="ExternalInput")  # [shalf, s, o]
    y = nc.dram_tensor("y", [BLOC, T, D], F32, kind="ExternalOutput")

    u_r = u[:].rearrange("b k p (j t) -> b j p k t", t=TT)
    # t = j*TT + c*SC + s*128 + p
    y_r = y[:].rearrange(
        "b (j c s p) o -> b j c p s o", p=128, s=SC // 128, c=NTH
    )

    mult = mybir.AluOpType.mult
    add = mybir.AluOpType.add

    with tile.TileContext(nc) as tc:
        with (
            tc.tile_pool(name="const", bufs=1) as const,
            tc.tile_pool(name="ut", bufs=BLOC * NJ) as ut_pool,
            tc.tile_pool(name="ysb", bufs=4) as ysb_pool,
            tc.tile_pool(name="hpool", bufs=1) as hpool,
            tc.tile_pool(name="ps_bu", bufs=2, space="PSUM") as ps_bu,
            tc.tile_pool(name="ps_y", bufs=4, space="PSUM") as ps_y,
        ):
            # First u supertile is the head-critical transfer: issue it
            # before everything else on the sync queue.
            uts = {}
            for b in range(BLOC):
                for j in range(NJ):
                    uts[(b, j)] = ut_pool.tile(
                        [128, 2, TT], BF16, tag="uT", name="uT"
                    )
            # Tiny consts first (they gate A_bc / scan init / LDWEIGHTS),
            # batched into single DMAs, then the first u half-supertile.
            Ah = const.tile([128, 4], F32, name="Ah")
            nc.sync.dma_start(Ah, Ahd[:])
            B_sb = const.tile([128, 2, D], BF16, name="B_sb")
            C_sb = const.tile([128, 2, D], BF16, name="C_sb")
            # First chunk's first segment (t<512) lands first; B (tiny,
            # needed by the first LDWEIGHTS) right behind it.
            nc.sync.dma_start(uts[(0, 0)][:, :, 0:512], u_r[0, 0][:, :, 0:512])
            nc.sync.dma_start(B_sb, Bd[:].rearrange("k i s -> i k s"))
            nc.sync.dma_start(uts[(0, 0)][:, :, 512:SC], u_r[0, 0][:, :, 512:SC])
            nc.sync.dma_start(uts[(0, 0)][:, :, SC:TT], u_r[0, 0][:, :, SC:TT])
            nc.sync.dma_start(C_sb, Cd[:].rearrange("k i s -> i k s"))
            for b in range(BLOC):
                for j in range(NJ):
                    if (b, j) != (0, 0):
                        nc.sync.dma_start(uts[(b, j)], u_r[b, j])

            # A_bc built on DVE (idle before the scan spine); keeps ACT's
            # first-use table load off the critical path.
            ones = const.tile([128, SC], F32, name="ones")
            nc.vector.memset(ones, 1.0)
            A_bc = const.tile([128, 2, SC], F32, name="A_bc")
            for m in range(2):
                nc.vector.tensor_scalar(
                    A_bc[:, m], ones, Ah[:, m : m + 1], None, op0=mult
                )

            # hidden states, [128s, b, mhalf, t]; persistent
            hT = hpool.tile([128, BLOC, 2, T], BF16, name="hT")

            # t = s0 + s*128 + p
            y_r2 = y[:].rearrange("b (s p) o -> b p s o", p=128)

            def emit_y(b_, s0_, seg_, copy_eng, dma_split=False):
                """y MMs + PSUM->SBUF copies + DMA out for one segment.
                dma_split issues the DMA per 256-t half (shorter exposed
                transfer at the kernel tail)."""
                y_sb = ysb_pool.tile(
                    [128, (seg_ // 128) * D], F32, tag="y_sb", name="y_sb"
                )
                y_sb_r = y_sb.rearrange("p (s o) -> p s o", s=seg_ // 128)
                for half in range(seg_ // 256):
                    y_ps = ps_y.tile(
                        [128, 2 * D], F32, tag="y_ps", name="y_ps"
                    )
                    for i in range(2):
                        t0 = s0_ + (half * 2 + i) * 128
                        for k in range(2):
                            nc.tensor.matmul(
                                y_ps[:, i * D : (i + 1) * D],
                                hT[:, b_, k, t0 : t0 + 128],
                                C_sb[:, k],
                                start=(k == 0),
                                stop=(k == 1),
                            )
                    copy_eng(
                        y_sb[:, half * 2 * D : (half + 1) * 2 * D], y_ps
                    )
                    if dma_split:
                        nc.sync.dma_start(
                            y_r2[b_][:, s0_ // 128 + half * 2
                                     : s0_ // 128 + half * 2 + 2],
                            y_sb_r[:, half * 2 : half * 2 + 2],
                        )
                if not dma_split:
                    nsub = seg_ // 128
                    nc.sync.dma_start(
                        y_r2[b_][:, s0_ // 128 : s0_ // 128 + nsub],
                        y_sb_r,
                    )

            # Software-pipelined: each chunk's Bu matmuls are emitted (and
            # thus prioritized on PE) BEFORE the previous segment's y-phase,
            # so the scan spine never waits on Bu behind y work.
            pending = []
            for b in range(BLOC):
                for j in range(NJ):
                    uT = uts[(b, j)]
                    for th in range(NTH):
                        c0 = j * TT + th * SC  # chunk start (abs time)
                        bu_tiles = []
                        for m in range(2):
                            bu_ps = ps_bu.tile(
                                [128, SC], F32, tag="bu_ps", name="bu_ps"
                            )
                            bu_tiles.append(bu_ps)
                            for hh in range(SC // MMF):
                                for k in range(2):
                                    nc.tensor.matmul(
                                        bu_ps[:, hh * MMF : (hh + 1) * MMF],
                                        B_sb[:, k, m * 128 : (m + 1) * 128],
                                        uT[:, k,
                                           th * SC + hh * MMF
                                           : th * SC + (hh + 1) * MMF],
                                        start=(k == 0),
                                        stop=(k == 1),
                                    )
                        for p in pending:
                            emit_y(*p, nc.scalar.copy)
                        pending = []
                        # First chunk: 2 segments so the spine starts on
                        # half the data.  Final chunk: 2 segments so the
                        # last y-phase overlaps the spine's tail.
                        last = b == BLOC - 1 and j == NJ - 1 and th == NTH - 1
                        first = b == 0 and j == 0 and th == 0
                        nseg = 2 if (last or first) else 1
                        seg = SC // nseg
                        for sg in range(nseg):
                            s0 = c0 + sg * seg
                            for m in range(2):
                                init = (
                                    Ah[:, 2 + m : 3 + m]
                                    if s0 == 0
                                    else hT[:, b, m, s0 - 1 : s0]
                                )
                                nc.vector.tensor_tensor_scan(
                                    hT[:, b, m, s0 : s0 + seg],
                                    A_bc[:, m, :seg],
                                    bu_tiles[m][:, s0 - c0 : s0 - c0 + seg],
                                    init,
                                    op0=mult,
                                    op1=add,
                                )
                            pending.append((b, s0, seg))
            # Tail: final segments' copies split ACT / DVE (DVE is idle),
            # DMAs issued per half so the last transfer is short.
            for idx, p in enumerate(pending):
                emit_y(
                    *p,
                    nc.vector.tensor_copy if idx else nc.scalar.copy,
                    dma_split=True,
                )

    nc.compile()
    return nc


def _get_nc():
    if "nc" not in _CACHE:
        _CACHE["nc"] = _build_nc()
    return _CACHE["nc"]


def make_in_maps(inputs, A, B, C, h0):
    u = np.asarray(inputs, dtype=np.float32).astype(ml_dtypes.bfloat16)
    # [B, T, 256] -> [B, 2, 128, T]  (i = k*128 + p)
    u = np.ascontiguousarray(u.transpose(0, 2, 1)).reshape(BATCH, 2, 128, T)
    A2 = np.asarray(A, np.float32).reshape(2, 128).T
    h02 = np.asarray(h0, np.float32).reshape(2, 128).T
    Ah0 = np.ascontiguousarray(np.concatenate([A2, h02], axis=1))  # [128, 4]
    Br = np.ascontiguousarray(
        np.asarray(B, np.float32).reshape(2, 128, D).astype(ml_dtypes.bfloat16)
    )
    Cr = np.ascontiguousarray(
        np.asarray(C, np.float32).reshape(2, 128, D).astype(ml_dtypes.bfloat16)
    )
    return [
        {
            "u": np.ascontiguousarray(u[c * BLOC : (c + 1) * BLOC]),
            "Ah0": Ah0,
            "B": Br,
            "C": Cr,
        }
        for c in range(NCORES)
    ]


def kernel(inputs, A, B, C, h0, _trace=False):
    nc = _get_nc()
    in_maps = make_in_maps(inputs, A, B, C, h0)
    res = bass_utils.run_bass_kernel_spmd(
        nc, in_maps, core_ids=list(range(NCORES)), trace=_trace
    )
    out = np.concatenate([r["y"] for r in res.results], axis=0)
    if _trace:
        _CACHE["last_result"] = res
    return out
